# revision 18
# baseline (speedup 1.0000x reference)
"""Trainium2 Bass kernel for LIFNet (leaky-integrator net, no spiking).

Math: the module is linear, and the leaky integration L (a causal LTI filter
along T) commutes with the per-timestep linear layers:

    V2 = L(L(batch @ W1^T) @ W2^T) = (L^2)(batch @ (W2 @ W1)^T)

with Wc = W2 @ W1 of shape [10, 784].  L^2 has impulse response
h[m] = beta^2 (m-1) alpha^(m-2) (m >= 2), which decays below f32 noise by
lag ~128, so the filter is applied as a banded blocked matmul with two
constant 128x128 blocks (intra-block R0, previous-block R1).

Sharding (balanced, max-core bytes minimized): each core gets 12 full b's
(cores 0-7 -> b 12c..12c+11, covering b 0..95) plus HALF (by T) of one of
the remaining b's 96..99: core c processes b 96+c//2, T-half c%2, as a
1152-t segment (128 warm-up t's for the upper half; the filter impulse
response is < 1e-16 beyond lag ~228, so starting the recursion 128 t's
early is exact to f32).

Device work per core (the stream is HBM-read bound, so x is fp8-e3m4,
host-encoded at 2x scale -- measured end-to-end rel err ~1.4e-2 vs the
2e-2 gate; weights stay bf16, the PE supports mixed bf16xfp8 exactly):
  - one SWDGE DMA per b ([112 part, 14 KB contiguous lines]); the first
    b's DMA is issued BEFORE the two packed const DMAs so the const
    descriptor generation overlaps the first bulk transfer.
  - z^T = Wc @ x^T via PE matmuls: per 500-t unit, all 7 d-chunks
    (K=112) accumulate into ONE PSUM quadrant (rows 32q..32q+9 of a
    [106, 500] bank tile, tile_position=(0, 32q), q = unit%4 so up to 4
    units' chains interleave on the array); the PSUM band is copied
    (f32->fp16) straight into the z^T staging tile -- no selector
    matmul, no intermediate stacking copies.
  - b's are processed in bands of 4, packed at 10-partition offsets
    (rows 10*(b%4)..+10) in the staging tile [40, 2048], so the PE
    transpose ([40,128]->[128,40]) and the banded filter matmuls
    (M=40) amortize over 4 b's and the band's output leaves as a
    single [40, 2000] fp16 DMA (deferred until all input DMAs are
    queued).  The T-segment forms a final narrow (PW=10, 9-block)
    group so the end-of-stream critical path is minimal; each band's
    stage-2/3 is emitted after the 2nd b of the NEXT band (the PE
    stream is in-order, so emission order controls head-of-line
    blocking).
  - All constants load over the same SWDGE FIFO as the bulk input
    (HWDGE queues are starved while the SWDGE queue is nonempty on
    trn2); V2^T band slices DMA out on the scalar HWDGE queue.
  - Host re-assembles [100, 2000, 10].
"""

import sys

import numpy as np

for _p in ("/opt/trn_rl_repo",):
    if _p not in sys.path:
        sys.path.append(_p)

B, T, DIN, H1, H2 = 100, 2000, 784, 100, 10
ALPHA, BETA = 0.7, 0.3

NCORES = 8
BPF = 12            # full b's per core (8 * 12 = 96)
BGRP = 4            # b's per stage-2/3 band (10-partition offsets)
PW = BGRP * H2      # 40: partition width of band stage-2/3
DC = 112            # d-chunk width (784 = 7 * 112), partition dim of x tiles
NDC = DIN // DC     # 7
XS = 2.0            # host pre-scale of x before fp8-e3m4 encode
TG = 500            # t-columns per z-matmul unit (PSUM bank max 512 f32)
NTG = T // TG       # 4
TB = 128            # t'-block for the filter stage
NTB = (T + TB - 1) // TB  # 16
TPADF = NTB * TB    # 2048 free-dim padding for the z^T staging buffer
TS = 1152           # segment length (9 t-blocks): 1024 lower / 128 warm-up
NTBS = TS // TB     # 9
SGU = 288           # segment z-matmul unit width (4 * 288 = 1152)
NSG = TS // SGU     # 4
SEG_LO = 1024       # lower-half cores emit t < 1024
SEG_W0 = TB         # upper-half warm-up t's (discarded)
RHF = 2 * TB        # rh cols in the packed const
CF = RHF + TB       # packed const free size (rh | eye-128)

_CACHE: dict = {}


def _filter_blocks() -> np.ndarray:
    """R = [R1 | R0] as [128, 256] f32: rhs blocks for the filter matmuls.

    out[o, t'] += sum_tl z_block[tl, o] * R[tl, t'] with R[tl, t'] =
    h[lag], lag = (t' - tl) + 128 for R1 (z from previous t-block) and
    (t' - tl) for R0 (intra-block, strictly causal).
    """
    m = np.arange(512, dtype=np.float64)
    h = np.zeros(512)
    h[2:] = BETA * BETA * (m[2:] - 1.0) * ALPHA ** (m[2:] - 2.0)
    tl = np.arange(TB)[:, None]
    tp = np.arange(TB)[None, :]
    r1 = h[tp - tl + TB]
    lag0 = tp - tl
    r0 = np.where(lag0 >= 2, h[np.clip(lag0, 0, None)], 0.0)
    return np.concatenate([r1, r0], axis=1).astype(np.float32)


def _build():
    """Build + compile the per-core Bass kernel (shared by all 8 cores)."""
    from contextlib import ExitStack

    import concourse.tile as tile
    from concourse import bacc, mybir

    f32 = mybir.dt.float32
    bf16 = mybir.dt.bfloat16
    fp16 = mybir.dt.float16
    fp8 = mybir.dt.float8e3
    nc = bacc.Bacc(
        "TRN2", target_bir_lowering=False, debug=False, num_devices=NCORES
    )

    xT = nc.dram_tensor("xT", [BPF, DC, NDC, T], fp8, kind="ExternalInput")
    xS = nc.dram_tensor("xS", [2, DC, NDC, TS // 2], fp8, kind="ExternalInput")
    wct = nc.dram_tensor("wct", [DC, NDC * H2], bf16, kind="ExternalInput")
    rheye = nc.dram_tensor("rheye", [TB, CF], fp16, kind="ExternalInput")
    vout = nc.dram_tensor(
        "vout", [(BPF + 1) * H2, T], fp16, kind="ExternalOutput"
    )

    with tile.TileContext(nc) as tc, ExitStack() as ctx:
        const = ctx.enter_context(tc.tile_pool(name="const", bufs=1))
        xpool = ctx.enter_context(tc.tile_pool(name="xp", bufs=6))
        xspool = ctx.enter_context(tc.tile_pool(name="xs", bufs=2))
        ring = ctx.enter_context(tc.tile_pool(name="ring", bufs=1))
        zbp = ctx.enter_context(tc.tile_pool(name="zbp", bufs=2))
        vsb = ctx.enter_context(tc.tile_pool(name="vsb", bufs=3))
        zps = ctx.enter_context(tc.tile_pool(name="zps", bufs=2, space="PSUM"))
        tpsum = ctx.enter_context(tc.tile_pool(name="tps", bufs=2, space="PSUM"))
        vpsum = ctx.enter_context(tc.tile_pool(name="vps", bufs=2, space="PSUM"))

        # Bulk input rides the sync HWDGE queue; consts ride SWDGE
        # (gpsimd) concurrently, and the SWDGE queue stays empty for
        # the rest of the stream so the deferred output writes drain
        # at full rate the moment they are ready.
        xt0 = xpool.tile([DC, NDC * T], fp8, tag="xt")
        nc.sync.dma_start(
            xt0[:].rearrange("p (c t) -> p c t", c=NDC), xT.ap()[0]
        )
        wct_sb = const.tile([DC, NDC * H2], bf16, tag="wct")
        nc.gpsimd.dma_start(wct_sb[:], wct.ap())
        rheye_sb = const.tile([TB, CF], fp16, tag="rheye")
        nc.gpsimd.dma_start(rheye_sb[:], rheye.ap())

        # Two-deep manual ring of z^T staging tiles.  Bands live at
        # 32-partition offsets (compute-engine partition bases must be
        # 32-aligned); rows 32q+10..31 and the t-pad cols must stay
        # zero (the full-width transpose contracts over all 128 rows),
        # memset once.
        # PE HAM warm-up: the clock gate releases (2x clock) only after
        # a few us of sustained matmul activity, so burn the initial
        # DMA wait on dummy matmuls over a zeroed scratch.
        warm = const.tile([TB, TG], bf16, tag="warm")
        nc.vector.memset(warm[:], 0.0)

        zts_ring = []
        for i in range(2):
            zt = ring.tile([TB, TPADF], fp16, tag=f"zts{i}", name=f"zts{i}")
            nc.vector.memset(zt[:], 0.0)
            zts_ring.append(zt)

        eng = [0]

        def alt_copy(dst, src):
            # alternate PSUM->SBUF copies between the two copy engines
            if eng[0] % 2 == 0:
                nc.scalar.copy(dst, src)
            else:
                nc.vector.tensor_copy(dst, src)
            eng[0] += 1

        def zchains(zts, row0, parts, pos0=0):
            """Interleaved z-matmul unit chains: parts = per-unit
            (xt, xoff, w, cstride, toff).  The 7 d-chunks of every unit
            accumulate into PSUM band rows 32q..32q+9 (q = pos0+unit,
            tile_position=(0, 32q)); chunk MMs are emitted c-outer so
            LDWEIGHTS at one array position overlaps streaming at
            another.  Bands are then copied (f32->fp16) straight into
            the z^T staging tile -- no selector matmul."""
            zp = zps.tile([3 * 32 + H2, TG], f32, tag="zp")
            for c in range(NDC):
                for u, (xt, xoff, w, cstride, _) in enumerate(parts):
                    q = pos0 + u
                    nc.tensor.matmul(
                        zp[32 * q : 32 * q + H2, 0:w],
                        wct_sb[:, c * H2 : (c + 1) * H2],
                        xt[:, xoff + c * cstride : xoff + c * cstride + w],
                        start=(c == 0),
                        stop=(c == NDC - 1),
                        tile_position=(0, 32 * q),
                    )
            for u, (_, _, w, _, toff) in enumerate(parts):
                q = pos0 + u
                alt_copy(
                    zts[row0 : row0 + H2, toff : toff + w],
                    zp[32 * q : 32 * q + H2, 0:w],
                )



        def stage1(b, bq, zts):
            xt = xt0 if b == 0 else xpool.tile([DC, NDC * T], fp8, tag="xt")
            if b != 0:
                nc.sync.dma_start(
                    xt[:].rearrange("p (c t) -> p c t", c=NDC), xT.ap()[b]
                )
            zchains(
                zts, 32 * bq,
                [(xt, u * TG, TG, T, u * TG) for u in range(NTG)],
            )

        def stage23(bs0, zts, ntb, tw, pw):
            """Transpose the staging tile per 128-t-block and apply the
            banded filter, whole band (pw partitions) at once."""
            zb = zbp.tile([TB, NTB * PW], fp16, tag="zb")
            for j in range(ntb):
                ztp = tpsum.tile([TB, TB], fp16, tag="ztp")
                if pw == PW:
                    # full-width transpose (bands live at 32-offsets;
                    # spare rows are zero), then compact 4x10 of the
                    # 128 columns into zb's dense 40 via a strided AP.
                    nc.tensor.transpose(
                        ztp[:],
                        zts[0:TB, j * TB : (j + 1) * TB],
                        rheye_sb[0:TB, RHF : RHF + TB],
                    )
                    alt_copy(
                        zb[:, j * PW : (j + 1) * PW].rearrange(
                            "p (g c) -> p g c", g=BGRP
                        ),
                        ztp[:].rearrange("p (g c) -> p g c", g=BGRP)[
                            :, :, 0:H2
                        ],
                    )
                else:
                    nc.tensor.transpose(
                        ztp[:, 0:pw],
                        zts[0:pw, j * TB : (j + 1) * TB],
                        rheye_sb[0:pw, RHF : RHF + pw],
                    )
                    alt_copy(zb[:, j * PW : j * PW + pw], ztp[:, 0:pw])

            v2 = vsb.tile([PW, TPADF], fp16, tag="v2")
            for j in range(ntb):
                vp = vpsum.tile([PW, TB], f32, tag="vp")
                n_mm = 2 if j > 0 else 1
                mm = 0
                for roff, jj in ((0, j - 1), (TB, j)):
                    if jj < 0:
                        continue
                    nc.tensor.matmul(
                        vp[0:pw, :],
                        zb[:, jj * PW : jj * PW + pw],
                        rheye_sb[:, roff : roff + TB],
                        start=(mm == 0),
                        stop=(mm == n_mm - 1),
                    )
                    mm += 1
                w = min(TB, tw - j * TB)
                alt_copy(v2[0:pw, j * TB : j * TB + w], vp[0:pw, 0:w])
            outq.append((v2, bs0, tw, pw))

        def flush_outs():
            for v2d, bs0, otw, opw in outq:
                nc.gpsimd.dma_start(
                    vout.ap()[bs0 * H2 : bs0 * H2 + opw, 0:otw],
                    v2d[0:opw, :otw],
                )
            outq.clear()

        def seg_pipeline(zts, xh):
            """The final T-segment, fully pipelined: half-0 units run
            while half-1 streams, so only ~2 z-units + the narrow
            stage-2/3 remain after the last input byte lands."""

            def units(us):
                zchains(
                    zts, 0,
                    [
                        (xh[u // 2], (u % 2) * SGU, SGU, TS // 2, u * SGU)
                        for u in us
                    ],
                    pos0=us[0],
                )

            zb = zbp.tile([TB, NTB * PW], fp16, tag="zb")
            v2 = vsb.tile([PW, TPADF], fp16, tag="v2")

            def trans(jlo, jhi):
                for j in range(jlo, jhi):
                    ztp = tpsum.tile([TB, TB], fp16, tag="ztp")
                    nc.tensor.transpose(
                        ztp[:, 0:H2],
                        zts[0:H2, j * TB : (j + 1) * TB],
                        rheye_sb[0:H2, RHF : RHF + H2],
                    )
                    alt_copy(zb[:, j * PW : j * PW + H2], ztp[:, 0:H2])

            def filt(jlo, jhi):
                for j in range(jlo, jhi):
                    vp = vpsum.tile([PW, TB], f32, tag="vp")
                    n_mm = 2 if j > 0 else 1
                    mm = 0
                    for roff, jj in ((0, j - 1), (TB, j)):
                        if jj < 0:
                            continue
                        nc.tensor.matmul(
                            vp[0:H2, :],
                            zb[:, jj * PW : jj * PW + H2],
                            rheye_sb[:, roff : roff + TB],
                            start=(mm == 0),
                            stop=(mm == n_mm - 1),
                        )
                        mm += 1
                    w = min(TB, TS - j * TB)
                    alt_copy(v2[0:H2, j * TB : j * TB + w], vp[0:H2, 0:w])

            units([0, 1])   # half 0: z cols 0..576 -> t-blocks 0..3
            trans(0, 4)
            filt(0, 4)
            units([2, 3])   # half 1 (after its DMA): cols 576..1152
            trans(4, NTBS)
            filt(4, NTBS)
            nc.gpsimd.dma_start(
                vout.ap()[BPF * H2 : (BPF + 1) * H2, 0:TS], v2[0:H2, 0:TS]
            )

        for _ in range(8):
            wp = zps.tile([3 * 32 + H2, TG], f32, tag="zp")
            nc.tensor.matmul(
                wp[0:PW, 0:TG], warm[:, 0:PW], warm[:, 0:TG],
                start=True, stop=True,
            )

        # The T-segment band is LAST (its narrow 9-block stage-2/3 is
        # the cheapest possible end-of-stream chain); each band's
        # stage-2/3 is emitted after the 2nd b of the NEXT band, and
        # the deferred output DMAs are released once all bulk input
        # DMAs are issued.
        bands = [
            list(range(k * BGRP, (k + 1) * BGRP)) for k in range(BPF // BGRP)
        ] + ["SEG"]
        pending = None
        outq = []
        for gi, bs in enumerate(bands):
            zts = zts_ring[gi % 2]
            if bs == "SEG":
                xh = []
                for h in range(2):
                    xts_t = xspool.tile(
                        [DC, NDC * (TS // 2)], fp8, tag="xts"
                    )
                    nc.sync.dma_start(
                        xts_t[:].rearrange("p (c t) -> p c t", c=NDC),
                        xS.ap()[h],
                    )
                    xh.append(xts_t)
                # all bulk input DMAs are queued: release the first
                # bands' buffered outputs now (they fire as soon as the
                # scalar engine reaches them), the last band's right
                # after its stage-2/3.
                flush_outs()
                if pending is not None:
                    stage23(*pending)
                    pending = None
                flush_outs()
                seg_pipeline(zts, xh)
                continue
            for bq, b in enumerate(bs):
                stage1(b, bq, zts)
                if bq == 1 and pending is not None:
                    stage23(*pending)
                    pending = None
            pending = (bs[0], zts, NTB, T, PW)
        if pending is not None:
            stage23(*pending)

    nc.compile()
    return nc


def _prep_inputs(batch: np.ndarray, W1: np.ndarray, W2: np.ndarray):
    import ml_dtypes

    bf16 = ml_dtypes.bfloat16
    fp16 = np.float16
    fp8 = ml_dtypes.float8_e3m4
    wc = (W2.astype(np.float64) @ W1.astype(np.float64)).astype(np.float64)
    wc = wc / XS  # undo the host pre-scale of x
    # [112, 7*10]: wct[p, c*10+o] = Wc[o, 112c + p]
    wct = np.ascontiguousarray(
        wc.T.reshape(NDC, DC, H2).transpose(1, 0, 2).reshape(DC, NDC * H2)
    ).astype(bf16)
    rheye = np.zeros((TB, CF), np.float32)
    rheye[:, 0:RHF] = _filter_blocks()
    rheye[:, RHF:CF] = np.eye(TB, dtype=np.float32)
    rheye = rheye.astype(fp16)

    xq = (batch * np.float32(XS)).astype(fp8)  # one pass over the f32 data

    # full b's 0..95: [8, 12, 112, 7, 2000]: core, b, d%112 (partitions),
    # d-chunk, t
    xt = np.ascontiguousarray(
        xq[: NCORES * BPF]
        .reshape(NCORES, BPF, T, NDC, DC)
        .transpose(0, 1, 4, 3, 2)
    )

    # T-segments of b's 96..99: core c gets b 96+c//2, half c%2.
    # Lower half: t 0..1152 (host keeps t<1024).  Upper half: t
    # 896..2048 (first 128 are filter warm-up; host keeps t>=1024).
    seg = np.zeros((NCORES, TS, DIN), fp8)
    for c in range(NCORES):
        be = NCORES * BPF + c // 2
        if c % 2 == 0:
            seg[c] = xq[be, 0:TS]
        else:
            seg[c, 0 : T - (SEG_LO - SEG_W0)] = xq[be, SEG_LO - SEG_W0 :]
    xs = np.ascontiguousarray(
        seg.reshape(NCORES, 2, TS // 2, NDC, DC).transpose(0, 1, 4, 3, 2)
    )
    return xt, xs, wct, rheye


def kernel(batch: np.ndarray, W1: np.ndarray, W2: np.ndarray) -> np.ndarray:
    from concourse import bass_utils

    if "nc" not in _CACHE:
        _CACHE["nc"] = _build()
    nc = _CACHE["nc"]

    xt, xs, wct, rheye = _prep_inputs(batch, W1, W2)
    in_maps = [
        {"xT": xt[i], "xS": xs[i], "wct": wct, "rheye": rheye}
        for i in range(NCORES)
    ]
    res = bass_utils.run_bass_kernel_spmd(
        nc, in_maps, core_ids=list(range(NCORES)), **_CACHE.get("run_kwargs", {})
    )
    _CACHE["last_result"] = res

    out = np.empty((B, T, H2), np.float32)
    for c in range(NCORES):
        vo = res.results[c]["vout"].astype(np.float32)  # [130, 2000]
        out[c * BPF : (c + 1) * BPF] = (
            vo[: BPF * H2].reshape(BPF, H2, T).transpose(0, 2, 1)
        )
        segv = vo[BPF * H2 :]  # [10, 2000]; valid cols 0..TS
        be = NCORES * BPF + c // 2
        if c % 2 == 0:
            out[be, 0:SEG_LO] = segv[:, 0:SEG_LO].T
        else:
            out[be, SEG_LO:T] = segv[:, SEG_W0 : SEG_W0 + (T - SEG_LO)].T
    return out


# revision 23
# speedup vs baseline: 1.0088x; 1.0088x over previous
"""Trainium2 Bass kernel for LIFNet (leaky-integrator net, no spiking).

Math: the module is linear, and the leaky integration L (a causal LTI filter
along T) commutes with the per-timestep linear layers:

    V2 = L(L(batch @ W1^T) @ W2^T) = (L^2)(batch @ (W2 @ W1)^T)

with Wc = W2 @ W1 of shape [10, 784].  L^2 has impulse response
h[m] = beta^2 (m-1) alpha^(m-2) (m >= 2), which decays below f32 noise by
lag ~128, so the filter is applied as a banded blocked matmul with two
constant 128x128 blocks (intra-block R0, previous-block R1).

Sharding (balanced, max-core bytes minimized): each core gets 12 full b's
(cores 0-7 -> b 12c..12c+11, covering b 0..95) plus HALF (by T) of one of
the remaining b's 96..99: core c processes b 96+c//2, T-half c%2, as a
1152-t segment (128 warm-up t's for the upper half; the filter impulse
response is < 1e-16 beyond lag ~228, so starting the recursion 128 t's
early is exact to f32).

Device work per core (the stream is HBM-read bound, so x is fp8-e3m4,
host-encoded at 2x scale -- measured end-to-end rel err ~1.4e-2 vs the
2e-2 gate; weights stay bf16, the PE supports mixed bf16xfp8 exactly):
  - one SWDGE DMA per b ([112 part, 14 KB contiguous lines]); the first
    b's DMA is issued BEFORE the two packed const DMAs so the const
    descriptor generation overlaps the first bulk transfer.
  - z^T = Wc @ x^T via PE matmuls: per 500-t unit, all 7 d-chunks
    (K=112) accumulate into ONE PSUM quadrant (rows 32q..32q+9 of a
    [106, 500] bank tile, tile_position=(0, 32q), q = unit%4 so up to 4
    units' chains interleave on the array); the PSUM band is copied
    (f32->fp16) straight into the z^T staging tile -- no selector
    matmul, no intermediate stacking copies.
  - b's are processed in bands of 4, packed at 10-partition offsets
    (rows 10*(b%4)..+10) in the staging tile [40, 2048], so the PE
    transpose ([40,128]->[128,40]) and the banded filter matmuls
    (M=40) amortize over 4 b's and the band's output leaves as a
    single [40, 2000] fp16 DMA (deferred until all input DMAs are
    queued).  The T-segment forms a final narrow (PW=10, 9-block)
    group so the end-of-stream critical path is minimal; each band's
    stage-2/3 is emitted after the 2nd b of the NEXT band (the PE
    stream is in-order, so emission order controls head-of-line
    blocking).
  - All constants load over the same SWDGE FIFO as the bulk input
    (HWDGE queues are starved while the SWDGE queue is nonempty on
    trn2); V2^T band slices DMA out on the scalar HWDGE queue.
  - Host re-assembles [100, 2000, 10].
"""

import sys

import numpy as np

for _p in ("/opt/trn_rl_repo",):
    if _p not in sys.path:
        sys.path.append(_p)

B, T, DIN, H1, H2 = 100, 2000, 784, 100, 10
ALPHA, BETA = 0.7, 0.3

NCORES = 8
BPF = 12            # full b's per core (8 * 12 = 96)
BGRP = 4            # b's per stage-2/3 band (10-partition offsets)
PW = BGRP * H2      # 40: partition width of band stage-2/3
DC = 112            # d-chunk width (784 = 7 * 112), partition dim of x tiles
NDC = DIN // DC     # 7
XS = 2.0            # host pre-scale of x before fp8-e3m4 encode
TG = 500            # t-columns per z-matmul unit (PSUM bank max 512 f32)
NTG = T // TG       # 4
TB = 128            # t'-block for the filter stage
NTB = (T + TB - 1) // TB  # 16
TPADF = NTB * TB    # 2048 free-dim padding for the z^T staging buffer
TS = 1152           # segment length (9 t-blocks): 1024 lower / 128 warm-up
NTBS = TS // TB     # 9
SGU = 288           # segment z-matmul unit width (4 * 288 = 1152)
NSG = TS // SGU     # 4
SEG_LO = 1024       # lower-half cores emit t < 1024
SEG_W0 = TB         # upper-half warm-up t's (discarded)
RHF = 2 * TB        # rh cols in the packed const
CF = RHF + TB       # packed const free size (rh | eye-128)

_CACHE: dict = {}


def _filter_blocks() -> np.ndarray:
    """R = [R1 | R0] as [128, 256] f32: rhs blocks for the filter matmuls.

    out[o, t'] += sum_tl z_block[tl, o] * R[tl, t'] with R[tl, t'] =
    h[lag], lag = (t' - tl) + 128 for R1 (z from previous t-block) and
    (t' - tl) for R0 (intra-block, strictly causal).
    """
    m = np.arange(512, dtype=np.float64)
    h = np.zeros(512)
    h[2:] = BETA * BETA * (m[2:] - 1.0) * ALPHA ** (m[2:] - 2.0)
    tl = np.arange(TB)[:, None]
    tp = np.arange(TB)[None, :]
    r1 = h[tp - tl + TB]
    lag0 = tp - tl
    r0 = np.where(lag0 >= 2, h[np.clip(lag0, 0, None)], 0.0)
    return np.concatenate([r1, r0], axis=1).astype(np.float32)


def _build():
    """Build + compile the per-core Bass kernel (shared by all 8 cores)."""
    from contextlib import ExitStack

    import concourse.tile as tile
    from concourse import bacc, mybir

    f32 = mybir.dt.float32
    bf16 = mybir.dt.bfloat16
    fp16 = mybir.dt.float16
    fp8 = mybir.dt.float8e3
    nc = bacc.Bacc(
        "TRN2", target_bir_lowering=False, debug=False, num_devices=NCORES
    )

    xT = nc.dram_tensor(
        "xT", [BPF, 2, DC, NDC, T // 2], fp8, kind="ExternalInput"
    )
    xS = nc.dram_tensor("xS", [2, DC, NDC, TS // 2], fp8, kind="ExternalInput")
    wct = nc.dram_tensor("wct", [DC, NDC * H2], bf16, kind="ExternalInput")
    rheye = nc.dram_tensor("rheye", [TB, CF], fp16, kind="ExternalInput")
    vout = nc.dram_tensor(
        "vout", [(BPF + 1) * H2, T], fp16, kind="ExternalOutput"
    )

    with tile.TileContext(nc) as tc, ExitStack() as ctx:
        const = ctx.enter_context(tc.tile_pool(name="const", bufs=1))
        xpool = ctx.enter_context(tc.tile_pool(name="xp", bufs=6))
        xspool = ctx.enter_context(tc.tile_pool(name="xs", bufs=2))
        ring = ctx.enter_context(tc.tile_pool(name="ring", bufs=1))
        zbp = ctx.enter_context(tc.tile_pool(name="zbp", bufs=2))
        vsb = ctx.enter_context(tc.tile_pool(name="vsb", bufs=3))
        zps = ctx.enter_context(tc.tile_pool(name="zps", bufs=2, space="PSUM"))
        tpsum = ctx.enter_context(tc.tile_pool(name="tps", bufs=3, space="PSUM"))
        vpsum = ctx.enter_context(tc.tile_pool(name="vps", bufs=3, space="PSUM"))

        # Bulk input rides the sync HWDGE queue; consts ride SWDGE
        # (gpsimd) concurrently, and the SWDGE queue stays empty for
        # the rest of the stream so the deferred output writes drain
        # at full rate the moment they are ready.
        xt0 = xpool.tile([DC, NDC * T], fp8, tag="xt")
        for h in range(2):
            nc.sync.dma_start(
                xt0[:, h * NDC * (T // 2) : (h + 1) * NDC * (T // 2)]
                .rearrange("p (c t) -> p c t", c=NDC),
                xT.ap()[0, h],
            )
        wct_sb = const.tile([DC, NDC * H2], bf16, tag="wct")
        nc.gpsimd.dma_start(wct_sb[:], wct.ap())
        rheye_sb = const.tile([TB, CF], fp16, tag="rheye")
        nc.gpsimd.dma_start(rheye_sb[:], rheye.ap())

        # Two-deep manual ring of z^T staging tiles.  Bands live at
        # 32-partition offsets (compute-engine partition bases must be
        # 32-aligned); rows 32q+10..31 and the t-pad cols must stay
        # zero (the full-width transpose contracts over all 128 rows),
        # memset once.
        # PE HAM warm-up: the clock gate releases (2x clock) only after
        # a few us of sustained matmul activity, so burn the initial
        # DMA wait on dummy matmuls over a zeroed scratch.
        warm = const.tile([TB, TG], bf16, tag="warm")
        nc.vector.memset(warm[:], 0.0)

        zts_ring = []
        for i in range(2):
            zt = ring.tile([TB, TPADF], fp16, tag=f"zts{i}", name=f"zts{i}")
            nc.vector.memset(zt[:], 0.0)
            zts_ring.append(zt)

        eng = [0]

        def alt_copy(dst, src):
            # alternate PSUM->SBUF copies between the two copy engines
            if eng[0] % 2 == 0:
                nc.scalar.copy(dst, src)
            else:
                nc.vector.tensor_copy(dst, src)
            eng[0] += 1

        def zchains(zts, row0, parts, pos0=0):
            """Interleaved z-matmul unit chains: parts = per-unit
            (xt, xoff, w, cstride, toff).  The 7 d-chunks of every unit
            accumulate into PSUM band rows 32q..32q+9 (q = pos0+unit,
            tile_position=(0, 32q)); chunk MMs are emitted c-outer so
            LDWEIGHTS at one array position overlaps streaming at
            another.  Bands are then copied (f32->fp16) straight into
            the z^T staging tile -- no selector matmul."""
            zp = zps.tile([3 * 32 + H2, TG], f32, tag="zp")
            for c in range(NDC):
                for u, (xt, xoff, w, cstride, _) in enumerate(parts):
                    q = pos0 + u
                    nc.tensor.matmul(
                        zp[32 * q : 32 * q + H2, 0:w],
                        wct_sb[:, c * H2 : (c + 1) * H2],
                        xt[:, xoff + c * cstride : xoff + c * cstride + w],
                        start=(c == 0),
                        stop=(c == NDC - 1),
                        tile_position=(0, 32 * q),
                    )
            for u, (_, _, w, _, toff) in enumerate(parts):
                q = pos0 + u
                alt_copy(
                    zts[row0 : row0 + H2, toff : toff + w],
                    zp[32 * q : 32 * q + H2, 0:w],
                )



        HF = NDC * (T // 2)  # free offset of t-half 1 in an xt tile

        def xoff_u(u):
            # unit u's free-dim base in the [p, (h c t)] xt layout
            return (u // 2) * HF + (u % 2) * TG

        def stage1(b, bq, zts, xt=None):
            if xt is None:
                xt = xpool.tile([DC, NDC * T], fp8, tag="xt")
                for h in range(2):
                    nc.sync.dma_start(
                        xt[:, h * HF : (h + 1) * HF].rearrange(
                            "p (c t) -> p c t", c=NDC
                        ),
                        xT.ap()[b, h],
                    )
            zchains(
                zts, 32 * bq,
                [(xt, xoff_u(u), TG, T // 2, u * TG) for u in range(NTG)],
            )

        def stage23_ops(bs0, zts, ntb, tw, pw):
            """Build per-t-block op closures for the transpose + banded
            filter of one band (pw partitions): returns (trans, filt,
            write) where trans[j]/filt[j] emit block j's instructions.
            Emitting them lets the end-game weave two bands' latency
            chains together on the in-order PE."""
            zb = zbp.tile([TB, NTB * PW], fp16, tag="zb")
            v2 = vsb.tile([PW, TPADF], fp16, tag="v2")

            def trans_j(j):
                ztp = tpsum.tile([TB, TB], fp16, tag="ztp")
                if pw == PW:
                    # full-width transpose (bands live at 32-offsets;
                    # spare rows are zero), then compact 4x10 of the
                    # 128 columns into zb's dense 40 via a strided AP.
                    nc.tensor.transpose(
                        ztp[:],
                        zts[0:TB, j * TB : (j + 1) * TB],
                        rheye_sb[0:TB, RHF : RHF + TB],
                    )
                    alt_copy(
                        zb[:, j * PW : (j + 1) * PW].rearrange(
                            "p (g c) -> p g c", g=BGRP
                        ),
                        ztp[:].rearrange("p (g c) -> p g c", g=BGRP)[
                            :, :, 0:H2
                        ],
                    )
                else:
                    nc.tensor.transpose(
                        ztp[:, 0:pw],
                        zts[0:pw, j * TB : (j + 1) * TB],
                        rheye_sb[0:pw, RHF : RHF + pw],
                    )
                    alt_copy(zb[:, j * PW : j * PW + pw], ztp[:, 0:pw])

            def filt_j(j):
                vp = vpsum.tile([PW, TB], f32, tag="vp")
                n_mm = 2 if j > 0 else 1
                mm = 0
                for roff, jj in ((0, j - 1), (TB, j)):
                    if jj < 0:
                        continue
                    nc.tensor.matmul(
                        vp[0:pw, :],
                        zb[:, jj * PW : jj * PW + pw],
                        rheye_sb[:, roff : roff + TB],
                        start=(mm == 0),
                        stop=(mm == n_mm - 1),
                    )
                    mm += 1
                w = min(TB, tw - j * TB)
                alt_copy(v2[0:pw, j * TB : j * TB + w], vp[0:pw, 0:w])

            def write():
                nc.gpsimd.dma_start(
                    vout.ap()[bs0 * H2 : bs0 * H2 + pw, 0:tw],
                    v2[0:pw, 0:tw],
                )

            trans = [lambda j=j: trans_j(j) for j in range(ntb)]
            filt = [lambda j=j: filt_j(j) for j in range(ntb)]
            return trans, filt, write

        def stage23(bs0, zts, ntb, tw, pw):
            trans, filt, write = stage23_ops(bs0, zts, ntb, tw, pw)
            for op in trans:
                op()
            for op in filt:
                op()
            write()

        def weave(a, b):
            # round-robin two op lists (PE is in-order: alternating the
            # two latency chains hides each one's copy waits under the
            # other's matmuls)
            ia, ib = iter(a), iter(b)
            out = []
            while True:
                done = True
                for it in (ia, ib):
                    op = next(it, None)
                    if op is not None:
                        done = False
                        op()
                if done:
                    break

        for _ in range(8):
            wp = zps.tile([3 * 32 + H2, TG], f32, tag="zp")
            nc.tensor.matmul(
                wp[0:PW, 0:TG], warm[:, 0:PW], warm[:, 0:TG],
                start=True, stop=True,
            )

        # Bands 0/1 run with the classic choreography (each band's
        # stage-2/3 emitted after the 2nd b of the next band, output
        # written immediately -- the SWDGE queue is otherwise empty so
        # writes fire the moment v2 is ready).  The last band and the
        # T-segment get a hand-scheduled end-game: b11 streams in two
        # halves, the band's stage-2/3 splits at the half boundary, and
        # the remaining blocks weave with the segment's narrow chain.
        bands = [list(range(k * BGRP, (k + 1) * BGRP)) for k in range(2)]
        pending = None
        for gi, bs in enumerate(bands):
            zts = zts_ring[gi % 2]
            for bq, b in enumerate(bs):
                if b == 0:
                    stage1(b, bq, zts, xt=xt0)
                else:
                    stage1(b, bq, zts)
                if bq == 1 and pending is not None:
                    stage23(*pending)
                    pending = None
            pending = (bs[0], zts, NTB, T, PW)

        # ---- end-game: band 2 (b8..b11) + segment ----
        zts2 = zts_ring[0]
        ztsS = zts_ring[1]
        for bq, b in enumerate((8, 9)):
            stage1(b, bq, zts2)
            if bq == 1:
                stage23(*pending)
                pending = None
        stage1(10, 2, zts2)
        # b11 in two half-transfers; the segment halves queue right after
        xt11 = []
        for h in range(2):
            xth = xspool.tile([DC, HF], fp8, tag="xth")
            nc.sync.dma_start(
                xth[:].rearrange("p (c t) -> p c t", c=NDC),
                xT.ap()[11, h],
            )
            xt11.append(xth)
        xh = []
        for h in range(2):
            xts_t = xspool.tile([DC, NDC * (TS // 2)], fp8, tag="xts")
            nc.sync.dma_start(
                xts_t[:].rearrange("p (c t) -> p c t", c=NDC), xS.ap()[h]
            )
            xh.append(xts_t)

        def seg_units(us):
            zchains(
                ztsS, 0,
                [
                    (xh[u // 2], (u % 2) * SGU, SGU, TS // 2, u * SGU)
                    for u in us
                ],
                pos0=us[0],
            )

        b2t, b2f, b2w = stage23_ops(8, zts2, NTB, T, PW)
        sgt, sgf, sgw = stage23_ops(BPF, ztsS, NTBS, TS, H2)

        # b11 half 0 -> band2 blocks 0..6 (t < 896) while half 1 streams
        zchains(
            zts2, 32 * 3,
            [(xt11[0], u * TG, TG, T // 2, u * TG) for u in (0, 1)],
            pos0=0,
        )
        for op in b2t[0:7]:
            op()
        for op in b2f[0:7]:
            op()
        # b11 half 1, then segment half 0 (its data lands just after)
        zchains(
            zts2, 32 * 3,
            [(xt11[1], (u - 2) * TG, TG, T // 2, u * TG) for u in (2, 3)],
            pos0=2,
        )
        seg_units([0, 1])
        # weave the band's remaining blocks with the segment's first 4
        weave(b2t[7:NTB] + b2f[7:NTB], sgt[0:4] + sgf[0:4])
        b2w()
        seg_units([2, 3])
        weave(sgt[4:NTBS], sgf[4:NTBS])
        sgw()

    nc.compile()
    return nc


def _prep_inputs(batch: np.ndarray, W1: np.ndarray, W2: np.ndarray):
    import ml_dtypes

    bf16 = ml_dtypes.bfloat16
    fp16 = np.float16
    fp8 = ml_dtypes.float8_e3m4
    wc = (W2.astype(np.float64) @ W1.astype(np.float64)).astype(np.float64)
    wc = wc / XS  # undo the host pre-scale of x
    # [112, 7*10]: wct[p, c*10+o] = Wc[o, 112c + p]
    wct = np.ascontiguousarray(
        wc.T.reshape(NDC, DC, H2).transpose(1, 0, 2).reshape(DC, NDC * H2)
    ).astype(bf16)
    rheye = np.zeros((TB, CF), np.float32)
    rheye[:, 0:RHF] = _filter_blocks()
    rheye[:, RHF:CF] = np.eye(TB, dtype=np.float32)
    rheye = rheye.astype(fp16)

    xq = (batch * np.float32(XS)).astype(fp8)  # one pass over the f32 data

    # full b's 0..95: [8, 12, 2, 112, 7, 1000]: core, b, t-half,
    # d%112 (partitions), d-chunk, t-within-half
    xt = np.ascontiguousarray(
        xq[: NCORES * BPF]
        .reshape(NCORES, BPF, 2, T // 2, NDC, DC)
        .transpose(0, 1, 2, 5, 4, 3)
    )

    # T-segments of b's 96..99: core c gets b 96+c//2, half c%2.
    # Lower half: t 0..1152 (host keeps t<1024).  Upper half: t
    # 896..2048 (first 128 are filter warm-up; host keeps t>=1024).
    seg = np.zeros((NCORES, TS, DIN), fp8)
    for c in range(NCORES):
        be = NCORES * BPF + c // 2
        if c % 2 == 0:
            seg[c] = xq[be, 0:TS]
        else:
            seg[c, 0 : T - (SEG_LO - SEG_W0)] = xq[be, SEG_LO - SEG_W0 :]
    xs = np.ascontiguousarray(
        seg.reshape(NCORES, 2, TS // 2, NDC, DC).transpose(0, 1, 4, 3, 2)
    )
    return xt, xs, wct, rheye


def kernel(batch: np.ndarray, W1: np.ndarray, W2: np.ndarray) -> np.ndarray:
    from concourse import bass_utils

    if "nc" not in _CACHE:
        _CACHE["nc"] = _build()
    nc = _CACHE["nc"]

    xt, xs, wct, rheye = _prep_inputs(batch, W1, W2)
    in_maps = [
        {"xT": xt[i], "xS": xs[i], "wct": wct, "rheye": rheye}
        for i in range(NCORES)
    ]
    res = bass_utils.run_bass_kernel_spmd(
        nc, in_maps, core_ids=list(range(NCORES)), **_CACHE.get("run_kwargs", {})
    )
    _CACHE["last_result"] = res

    out = np.empty((B, T, H2), np.float32)
    for c in range(NCORES):
        vo = res.results[c]["vout"].astype(np.float32)  # [130, 2000]
        out[c * BPF : (c + 1) * BPF] = (
            vo[: BPF * H2].reshape(BPF, H2, T).transpose(0, 2, 1)
        )
        segv = vo[BPF * H2 :]  # [10, 2000]; valid cols 0..TS
        be = NCORES * BPF + c // 2
        if c % 2 == 0:
            out[be, 0:SEG_LO] = segv[:, 0:SEG_LO].T
        else:
            out[be, SEG_LO:T] = segv[:, SEG_W0 : SEG_W0 + (T - SEG_LO)].T
    return out


# revision 26
# speedup vs baseline: 1.1681x; 1.1579x over previous
"""Trainium2 Bass kernel for LIFNet (leaky-integrator net, no spiking).

Math: the module is linear, and the leaky integration L (a causal LTI filter
along T) commutes with the per-timestep linear layers:

    V2 = L(L(batch @ W1^T) @ W2^T) = (L^2)(batch @ (W2 @ W1)^T)

with Wc = W2 @ W1 of shape [10, 784].  The double integration is evaluated
EXACTLY as two chained first-order recurrences on the Vector engine's
``tensor_tensor_scan`` (fp32 internal state):

    W[t]  = a*W[t-1]  + b^2 * z[t-1]      (W = b*V1)
    V2[t] = a*V2[t-1] + W[t-1]

so the Tensor engine runs nothing but the z-matmuls, whose 4-way
column-group overlap is preserved (no transpose / filter matmuls to
interleave), and the end-of-stream critical path is two short scan chunks.

Sharding (balanced, max-core bytes minimized): each core gets 12 full b's
(cores 0-7 -> b 12c..12c+11, covering b 0..95) plus HALF (by T) of one of
the remaining b's 96..99: core c processes b 96+c//2, T-half c%2, as a
1152-t segment (128 warm-up t's for the upper half; a^256 << 1e-30 so
starting the recursion 128 t's early is exact to f32).

Device work per core (the stream is HBM-read bound, so x is fp8-e3m4,
host-encoded at 2x scale -- measured end-to-end rel err ~1.3e-2 vs the
2e-2 gate; weights stay bf16, the PE supports mixed bf16 x fp8 exactly):
  - bulk input on the sync HWDGE queue, two 784 KB half-b transfers per b
    ([112 part, 7 KB contiguous lines]); the tiny wct const rides the
    otherwise-empty SWDGE queue concurrently with the first transfer.
  - z^T = Wc @ x^T per 500-t unit: all 7 d-chunks (K=112) accumulate into
    one PSUM band (rows 32q..32q+9 of a [106, 500] bank tile,
    tile_position=(0, 32q), q = unit%4); the four units of a b are
    emitted chunk-outer so LDWEIGHTS at one array position overlaps
    streaming at another and the 4 chains run concurrently.
  - the PSUM band is copied to the z^T staging tile by the Scalar engine
    with a fused *b^2 scale (f32 -> fp16).
  - per band of 4 b's (staging rows 32q..32q+9), the two scans run in
    500-col chunks chained via ``initial=prev[:, c-1:c]``, so each chunk
    fires as soon as the last b's unit-copy lands; V2 rows then DMA out
    per b on the SWDGE queue (empty mid-stream, so writes drain the
    moment they are ready).
  - the last b and the T-segment stream in half-transfers so only ~2
    z-units + two 500-col scan chunks + a 23 KB write remain after the
    last input byte.
  - Host re-assembles [100, 2000, 10].
"""

import sys

import numpy as np

for _p in ("/opt/trn_rl_repo",):
    if _p not in sys.path:
        sys.path.append(_p)

B, T, DIN, H1, H2 = 100, 2000, 784, 100, 10
ALPHA, BETA = 0.7, 0.3

NCORES = 8
BPF = 12            # full b's per core (8 * 12 = 96)
BGRP = 4            # b's per band (32-partition offsets in the staging tile)
DC = 112            # d-chunk width (784 = 7 * 112), partition dim of x tiles
NDC = DIN // DC     # 7
XS = 2.0            # host pre-scale of x before fp8-e3m4 encode
TG = 500            # t-columns per z-matmul unit (PSUM bank max 512 f32)
NTG = T // TG       # 4
TS = 1152           # segment length: 1024 lower / 128 warm-up
SGU = 288           # segment z-matmul unit width (4 * 288 = 1152)
SEG_LO = 1024       # lower-half cores emit t < 1024
SEG_W0 = 128        # upper-half warm-up t's (discarded)

_CACHE: dict = {}


def _build():
    """Build + compile the per-core Bass kernel (shared by all 8 cores)."""
    from contextlib import ExitStack

    import concourse.tile as tile
    from concourse import bacc, mybir

    f32 = mybir.dt.float32
    bf16 = mybir.dt.bfloat16
    fp16 = mybir.dt.float16
    fp8 = mybir.dt.float8e3
    nc = bacc.Bacc(
        "TRN2", target_bir_lowering=False, debug=False, num_devices=NCORES
    )

    xT = nc.dram_tensor(
        "xT", [BPF, 2, DC, NDC, T // 2], fp8, kind="ExternalInput"
    )
    xS = nc.dram_tensor("xS", [2, DC, NDC, TS // 2], fp8, kind="ExternalInput")
    wct = nc.dram_tensor("wct", [DC, NDC * H2], bf16, kind="ExternalInput")
    vout = nc.dram_tensor(
        "vout", [(BPF + 1) * H2, T], fp16, kind="ExternalOutput"
    )

    HF = NDC * (T // 2)  # free offset of t-half 1 in an xt tile

    with tile.TileContext(nc) as tc, ExitStack() as ctx:
        const = ctx.enter_context(tc.tile_pool(name="const", bufs=1))
        xpool = ctx.enter_context(tc.tile_pool(name="xp", bufs=6))
        xspool = ctx.enter_context(tc.tile_pool(name="xs", bufs=2))
        ring = ctx.enter_context(tc.tile_pool(name="ring", bufs=1))
        wpool = ctx.enter_context(tc.tile_pool(name="wp", bufs=2))
        vpool = ctx.enter_context(tc.tile_pool(name="vp", bufs=3))
        zps = ctx.enter_context(tc.tile_pool(name="zps", bufs=2, space="PSUM"))

        # First bulk DMA goes out on the sync HWDGE queue; the wct const
        # rides SWDGE (gpsimd) concurrently.  The SWDGE queue stays
        # empty for the rest of the stream so output writes drain the
        # moment they are ready.
        xt0 = xpool.tile([DC, NDC * T], fp8, tag="xt")
        for h in range(2):
            nc.sync.dma_start(
                xt0[:, h * HF : (h + 1) * HF].rearrange(
                    "p (c t) -> p c t", c=NDC
                ),
                xT.ap()[0, h],
            )
        wct_sb = const.tile([DC, NDC * H2], bf16, tag="wct")
        nc.gpsimd.dma_start(wct_sb[:], wct.ap())

        # alpha operand for the scans (data0 must be a tensor)
        alpha_sb = const.tile([128, T], f32, tag="alpha")
        nc.vector.memset(alpha_sb[:], ALPHA)

        # z^T staging ring: bands live at 32-partition offsets (compute
        # engines need 32-aligned partition bases); spare rows are never
        # read to any visible output (the scans are partition-parallel
        # and the out-DMA slices per b), so no zeroing is needed.
        zts_ring = []
        for i in range(2):
            zt = ring.tile([128, T], fp16, tag=f"zts{i}", name=f"zts{i}")
            zts_ring.append(zt)

        def zchains(zts, row0, parts, pos0=0):
            """Interleaved z-matmul unit chains: parts = per-unit
            (xt, xoff, w, toff).  The 7 d-chunks of every unit
            accumulate into PSUM band rows 32q..32q+9 (q = pos0+unit,
            tile_position=(0, 32q)); chunk MMs are emitted c-outer so
            LDWEIGHTS at one array position overlaps streaming at
            another.  Bands are then copied (f32 -> fp16, fused *b^2)
            into the z^T staging tile by the Scalar engine."""
            zp = zps.tile([3 * 32 + H2, TG], f32, tag="zp")
            for c in range(NDC):
                for u, (xt, xoff, w, cs, _) in enumerate(parts):
                    q = pos0 + u
                    nc.tensor.matmul(
                        zp[32 * q : 32 * q + H2, 0:w],
                        wct_sb[:, c * H2 : (c + 1) * H2],
                        xt[:, xoff + c * cs : xoff + c * cs + w],
                        start=(c == 0),
                        stop=(c == NDC - 1),
                        tile_position=(0, 32 * q),
                    )
            for u, (_, _, w, _, toff) in enumerate(parts):
                q = pos0 + u
                nc.scalar.mul(
                    zts[row0 : row0 + H2, toff : toff + w],
                    zp[32 * q : 32 * q + H2, 0:w],
                    BETA * BETA,
                )

        def stage1(b, bq, zts, xt=None):
            if xt is None:
                xt = xpool.tile([DC, NDC * T], fp8, tag="xt")
                for h in range(2):
                    nc.sync.dma_start(
                        xt[:, h * HF : (h + 1) * HF].rearrange(
                            "p (c t) -> p c t", c=NDC
                        ),
                        xT.ap()[b, h],
                    )
            zchains(
                zts, 32 * bq,
                [
                    (xt, (u // 2) * HF + (u % 2) * TG, TG, T // 2, u * TG)
                    for u in range(NTG)
                ],
            )

        def band_scans(bs0, zts, tw, bounds, nb):
            """Two chained scans (W then V2) over the staging tile, in
            chunks so each fires as soon as its z columns land, then the
            per-b output DMAs.  bounds = ascending chunk edges starting
            at 1, ending at tw; nb = b's in the band (rows 32g..32g+9
            hold b bs0+g)."""
            rows = 32 * (nb - 1) + H2
            w = wpool.tile([128, T], fp16, tag="w")
            v2 = vpool.tile([128, T], fp16, tag="v2")
            nc.vector.memset(w[0:rows, 0:1], 0.0)
            nc.vector.memset(v2[0:rows, 0:1], 0.0)
            mult = mybir.AluOpType.mult
            add = mybir.AluOpType.add
            for lo, hi in zip(bounds[:-1], bounds[1:]):
                for s, dd in ((zts, w), (w, v2)):
                    nc.vector.tensor_tensor_scan(
                        dd[0:rows, lo:hi],
                        alpha_sb[0:rows, lo:hi],
                        s[0:rows, lo - 1 : hi - 1],
                        0.0 if lo == 1 else dd[0:rows, lo - 1 : lo],
                        mult,
                        add,
                    )
            for g in range(nb):
                nc.gpsimd.dma_start(
                    vout.ap()[(bs0 + g) * H2 : (bs0 + g + 1) * H2, 0:tw],
                    v2[32 * g : 32 * g + H2, 0:tw],
                )

        BB = [1, TG + 1, 2 * TG + 1, 3 * TG + 1, T]  # band chunk edges

        # bands 0/1
        for gi in range(2):
            zts = zts_ring[gi % 2]
            for bq in range(BGRP):
                b = gi * BGRP + bq
                stage1(b, bq, zts, xt=xt0 if b == 0 else None)
            band_scans(gi * BGRP, zts, T, BB, BGRP)

        # ---- end-game: band 2 (b8..b11) + segment ----
        zts2 = zts_ring[0]
        ztsS = zts_ring[1]
        for bq, b in enumerate((8, 9, 10)):
            stage1(b, bq, zts2)
        # b11 and the segment stream in half-transfers
        xt11 = []
        for h in range(2):
            xth = xspool.tile([DC, HF], fp8, tag="xth")
            nc.sync.dma_start(
                xth[:].rearrange("p (c t) -> p c t", c=NDC), xT.ap()[11, h]
            )
            xt11.append(xth)
        xh = []
        for h in range(2):
            xts_t = xspool.tile([DC, NDC * (TS // 2)], fp8, tag="xts")
            nc.sync.dma_start(
                xts_t[:].rearrange("p (c t) -> p c t", c=NDC), xS.ap()[h]
            )
            xh.append(xts_t)
        zchains(
            zts2, 32 * 3,
            [(xt11[0], u * TG, TG, T // 2, u * TG) for u in (0, 1)],
            pos0=0,
        )
        zchains(
            zts2, 32 * 3,
            [(xt11[1], (u - 2) * TG, TG, T // 2, u * TG) for u in (2, 3)],
            pos0=2,
        )

        def seg_parts(us):
            return [
                (xh[u // 2], (u % 2) * SGU, SGU, TS // 2, u * SGU) for u in us
            ]

        zchains(ztsS, 0, seg_parts((0, 1)), pos0=0)
        zchains(ztsS, 0, seg_parts((2, 3)), pos0=2)
        band_scans(8, zts2, T, BB, BGRP)
        band_scans(BPF, ztsS, TS, [1, 2 * SGU + 1, TS], 1)

    nc.compile()
    return nc


def _prep_inputs(batch: np.ndarray, W1: np.ndarray, W2: np.ndarray):
    import ml_dtypes

    bf16 = ml_dtypes.bfloat16
    fp8 = ml_dtypes.float8_e3m4
    wc = W2.astype(np.float64) @ W1.astype(np.float64)
    wc = wc / XS  # undo the host pre-scale of x
    # [112, 7*10]: wct[p, c*10+o] = Wc[o, 112c + p]
    wct = np.ascontiguousarray(
        wc.T.reshape(NDC, DC, H2).transpose(1, 0, 2).reshape(DC, NDC * H2)
    ).astype(bf16)

    xq = (batch * np.float32(XS)).astype(fp8)  # one pass over the f32 data

    # full b's 0..95: [8, 12, 2, 112, 7, 1000]: core, b, t-half,
    # d%112 (partitions), d-chunk, t-within-half
    xt = np.ascontiguousarray(
        xq[: NCORES * BPF]
        .reshape(NCORES, BPF, 2, T // 2, NDC, DC)
        .transpose(0, 1, 2, 5, 4, 3)
    )

    # T-segments of b's 96..99: core c gets b 96+c//2, half c%2.
    # Lower half: t 0..1152 (host keeps t<1024).  Upper half: t
    # 896..2048 (first 128 are recursion warm-up; host keeps t>=1024).
    seg = np.zeros((NCORES, TS, DIN), fp8)
    for c in range(NCORES):
        be = NCORES * BPF + c // 2
        if c % 2 == 0:
            seg[c] = xq[be, 0:TS]
        else:
            seg[c, 0 : T - (SEG_LO - SEG_W0)] = xq[be, SEG_LO - SEG_W0 :]
    xs = np.ascontiguousarray(
        seg.reshape(NCORES, 2, TS // 2, NDC, DC).transpose(0, 1, 4, 3, 2)
    )
    return xt, xs, wct


def kernel(batch: np.ndarray, W1: np.ndarray, W2: np.ndarray) -> np.ndarray:
    from concourse import bass_utils

    if "nc" not in _CACHE:
        _CACHE["nc"] = _build()
    nc = _CACHE["nc"]

    xt, xs, wct = _prep_inputs(batch, W1, W2)
    in_maps = [
        {"xT": xt[i], "xS": xs[i], "wct": wct} for i in range(NCORES)
    ]
    res = bass_utils.run_bass_kernel_spmd(
        nc, in_maps, core_ids=list(range(NCORES)), **_CACHE.get("run_kwargs", {})
    )
    _CACHE["last_result"] = res

    out = np.empty((B, T, H2), np.float32)
    for c in range(NCORES):
        vo = res.results[c]["vout"].astype(np.float32)  # [130, 2000]
        out[c * BPF : (c + 1) * BPF] = (
            vo[: BPF * H2].reshape(BPF, H2, T).transpose(0, 2, 1)
        )
        segv = vo[BPF * H2 :]  # [10, 2000]; valid cols 0..TS
        be = NCORES * BPF + c // 2
        if c % 2 == 0:
            out[be, 0:SEG_LO] = segv[:, 0:SEG_LO].T
        else:
            out[be, SEG_LO:T] = segv[:, SEG_W0 : SEG_W0 + (T - SEG_LO)].T
    return out
